# revision 7
# baseline (speedup 1.0000x reference)
"""Trainium2 Bass kernel for a dense transformer block.

Reference computation (per batch element):
    y  = Attention(LN1(x)) ; x = x + y
    x  = x + MLP(LN2(x))
with B=8, N=1024, C=768, H=12 heads, head_dim=64, HIDDEN=3072, fp32 I/O.

Sharding: data-parallel over B across the 8 NeuronCores — each core runs the
full block on one (1024, 768) batch element with replicated weights. No
collectives.

Per-core design notes (v3):
  * Attention runs as a software-pipelined loop over 6 head PAIRS with a
    3-stage skew: iteration i emits AV+normalize(i-2), QKV(i), and
    scores+exp(i-1).  This keeps the PE fed while the Scalar engine
    (exp, the attention bottleneck at ~18µs/pair) streams softmax.
  * Scores for a head pair are row-packed: even head occupies PE rows 0-63,
    odd head rows 64-127 (tile_position row groups), so the two heads'
    K=64 score matmuls run CONCURRENTLY in the systolic array.
  * exp is issued at free-dim 1024 (one ACT instruction per (head, key-tile)
    reading a 2-bank PSUM tile) and writes fp8e4m3 probabilities.
  * QKV / AV / proj / fc1 all run as fp8 DoubleRow matmuls (contraction 256
    per instruction), halving their PE streaming time.  fc2 stays bf16: the
    output tolerance cannot absorb fp8 on both MLP matmuls.
  * fp8 scale management: wqkv/wproj/wfc1 are pre-scaled x8 on the host
    (their 0.02-magnitude entries would land in the fp8 subnormal range).
    q/k evictions fold in 1/8 so scores are true-scale; V keeps the x8 and
    the ones-column of the AV accumulator is set to 0.25 so the normalize
    step emits attnT at x32 (good fp8 range); proj's x8-weight then makes
    the proj output x256, folded back in the residual add; fc1's x8 is
    folded into the Gelu activation's scale input.
  * Softmax reciprocal uses reciprocal_approx_fast; all PSUM evictions run
    on the Vector engine so the Scalar engine only does exp/gelu/sqrt.
"""

import numpy as np
import ml_dtypes

import concourse.bass as bass
import concourse.bacc as bacc
import concourse.mybir as mybir
import concourse.tile as tile
from concourse import bass_utils

# Model dims (hardcoded per the problem spec).
B = 8
N = 1024  # tokens
C = 768  # model dim
H = 12  # heads
HD = 64  # head dim
HID = 3072  # mlp hidden
EPS = 1e-5
P = 128  # SBUF partitions

NT = N // P  # 8 token tiles
KC = C // P  # 6 contraction tiles over C
KH = HID // P  # 24 contraction tiles over HIDDEN
NPAIR = H // 2  # 6 head pairs

F32 = mybir.dt.float32
BF16 = mybir.dt.bfloat16
FP8 = mybir.dt.float8e4
AF = mybir.ActivationFunctionType
ALU = mybir.AluOpType
DR = mybir.MatmulPerfMode.DoubleRow

# Feature switches (fallbacks for debugging).
ATT_FP8 = True  # fp8 DoubleRow qkv + attention@V + proj
FC1_FP8 = True  # fp8 DoubleRow fc1

WSCALE = 8.0  # host-side scale on fp8 weights

_cache = {}


def _build(flags):
    """Trace the per-core Bass program. `flags` gates optional bias/gain work."""
    (use_bqkv, use_g1, use_beta1, use_g2, use_beta2, use_bfc1, use_bproj,
     use_bfc2) = flags

    a_dt = FP8 if ATT_FP8 else BF16
    x2_dt = FP8 if FC1_FP8 else BF16
    w1_dt = FP8 if FC1_FP8 else BF16

    nc = bacc.Bacc("TRN2", target_bir_lowering=False, debug=False)

    x_d = nc.dram_tensor("x", [N, C], F32, kind="ExternalInput")
    wqkv_d = nc.dram_tensor("wqkv", [C, 3 * C], a_dt, kind="ExternalInput")
    wproj_d = nc.dram_tensor("wproj", [C, C], a_dt, kind="ExternalInput")
    wfc1_d = nc.dram_tensor("wfc1", [C, HID], w1_dt, kind="ExternalInput")
    wfc2_d = nc.dram_tensor("wfc2", [HID, C], BF16, kind="ExternalInput")
    out_d = nc.dram_tensor("out", [N, C], F32, kind="ExternalOutput")

    opt_d = {}
    for name, use, shape in (
        ("bqkv", use_bqkv, [3 * C]),
        ("g1", use_g1, [C]),
        ("beta1", use_beta1, [C]),
        ("g2", use_g2, [C]),
        ("beta2", use_beta2, [C]),
        ("bfc1", use_bfc1, [HID]),
        ("bproj", use_bproj, [C]),
        ("bfc2", use_bfc2, [C]),
    ):
        if use:
            opt_d[name] = nc.dram_tensor(name, shape, F32, kind="ExternalInput")

    def bcast_from_dram(pool, ap_1d, n, name):
        """[n] DRAM vector -> [P, n] SBUF tile replicated on every partition."""
        t = pool.tile([P, n], F32, name=name)
        src = bass.AP(tensor=ap_1d.tensor, offset=ap_1d.offset,
                      ap=[[0, P]] + list(ap_1d.ap))
        nc.sync.dma_start(out=t, in_=src)
        return t

    with tile.TileContext(nc) as tc:
        persist = tc.alloc_tile_pool(name="persist", bufs=1, side="left")
        psum = tc.alloc_tile_pool(name="psum", bufs=1, space="PSUM")
        dram = tc.alloc_tile_pool(name="dram", bufs=2, space="DRAM")

        # Residual stream, token-major; updated in place through the block.
        x_sb = persist.tile([P, NT, C], F32)
        for t in range(NT):
            nc.sync.dma_start(out=x_sb[:, t, :], in_=x_d.ap()[t * P:(t + 1) * P, :])
        eps_t = persist.tile([P, 1], F32)
        nc.vector.memset(eps_t, EPS)

        # Identity (bf16, embedded in the NEFF) for PE-based transposes.
        ident_d = nc.inline_tensor(np.eye(P, dtype=ml_dtypes.bfloat16), "ident")
        ident = persist.tile([P, P], BF16)
        nc.sync.dma_start(out=ident, in_=ident_d.ap())

        # fc1 weights + LN2 output live in persist so the wfc1 DMA can run
        # during attention instead of serializing after it.
        wfc1_sb = persist.tile([P, KC, HID], w1_dt)
        nc.sync.dma_start(out=wfc1_sb,
                          in_=wfc1_d.ap().rearrange("(k p) m -> p k m", p=P))
        x2lnT = persist.tile([P, KC, N], x2_dt)

        g_beta = {}
        for name in ("g1", "beta1", "g2", "beta2", "bproj", "bfc2"):
            if name in opt_d:
                g_beta[name] = bcast_from_dram(persist, opt_d[name].ap(), C,
                                               f"bc_{name}")
        bqkv_sb = None
        if "bqkv" in opt_d:
            bqkv_sb = persist.tile([P, 3 * C // P], F32)
            nc.sync.dma_start(out=bqkv_sb,
                              in_=opt_d["bqkv"].ap().rearrange("(m p) -> p m", p=P))
            g_beta["bqkv_v"] = bcast_from_dram(
                persist, opt_d["bqkv"].ap()[2 * C:3 * C], C, "bc_bqkv_v")
        bfc1_sb = None
        if "bfc1" in opt_d:
            bfc1_sb = persist.tile([P, KH], F32)
            nc.sync.dma_start(out=bfc1_sb,
                              in_=opt_d["bfc1"].ap().rearrange("(m p) -> p m", p=P))

        # ---------------------------------------------------------------
        # Pools for phase 1+2
        # ---------------------------------------------------------------
        p1 = tc.alloc_tile_pool(name="p1", bufs=1, side="left")
        ln1 = tc.alloc_tile_pool(name="ln1", bufs=3, side="left")

        wqkv_sb = p1.tile([P, KC, 3 * C], a_dt)
        nc.sync.dma_start(out=wqkv_sb,
                          in_=wqkv_d.ap().rearrange("(k p) m -> p k m", p=P))

        xlnT = p1.tile([P, KC, N], a_dt)

        def layernorm_tile(pool, x_ap, g_sb, beta_sb, name):
            """x_ap: [P, C] fp32 token-major -> returns [P, C] bf16 tile."""
            stats = pool.tile([P, 3, 6], F32, tag=f"{name}_st", bufs=3)
            xr = x_ap.rearrange("p (s f) -> p s f", f=256)
            for s in range(3):
                nc.vector.bn_stats(out=stats[:, s, :], in_=xr[:, s, :])
            mv = pool.tile([P, 2], F32, tag=f"{name}_mv", bufs=3)
            nc.vector.bn_aggr(out=mv, in_=stats)
            rstd = pool.tile([P, 1], F32, tag=f"{name}_rs", bufs=3)
            nc.scalar.activation(out=rstd, in_=mv[:, 1:2], func=AF.Sqrt,
                                 bias=eps_t, scale=1.0)
            nc.vector.reciprocal(out=rstd, in_=rstd)
            xln = pool.tile([P, C], BF16, tag=f"{name}_xln", bufs=3)
            nc.vector.tensor_scalar(out=xln, in0=x_ap, scalar1=mv[:, 0:1],
                                    scalar2=rstd, op0=ALU.subtract, op1=ALU.mult)
            if g_sb is not None:
                nc.vector.tensor_mul(out=xln, in0=xln, in1=g_sb)
            if beta_sb is not None:
                nc.vector.tensor_add(out=xln, in0=xln, in1=beta_sb)
            return xln

        def transpose_to(xln, dstT, t):
            """[P, C] token-major tile -> dstT[:, :, t*P:(t+1)*P] feature-major."""
            for c in range(KC):
                tps = psum.tile([P, P], BF16, tag="av", bufs=2, name="tps")
                nc.tensor.transpose(tps, xln[:, c * P:(c + 1) * P], ident)
                nc.vector.tensor_copy(out=dstT[:, c, t * P:(t + 1) * P], in_=tps)

        # ---------------------------------------------------------------
        # Phase 2: attention superloop over head pairs, 3-stage pipeline:
        #   iter i:  AV+normalize(i-2) | QKV(i) | scores+exp(i-1)
        # ---------------------------------------------------------------
        p2 = tc.alloc_tile_pool(name="p2", bufs=1, side="right")
        att = tc.alloc_tile_pool(name="att", bufs=1, side="left")
        qkT = p2.tile([P, 2 * NPAIR, N], BF16)
        # V per head, token-tiles on dim2; slot HD is the denominator column
        # (0.25 with fp8 scaling so attnT comes out x32); slots HD+1.. pad the
        # kt stride to a multiple of 16 bytes (DoubleRow AP constraint).
        VW = 80 if ATT_FP8 else 72
        ONES = 0.25 if ATT_FP8 else 1.0
        v_aug = p2.tile([P, H, NT, VW], a_dt)
        nc.vector.memset(v_aug[:, :, :, HD:HD + 1], ONES)
        attnT = p2.tile([P, KC, N], a_dt)
        wproj_sb = p2.tile([P, KC, C], a_dt)
        nc.sync.dma_start(out=wproj_sb,
                          in_=wproj_d.ap().rearrange("(k p) m -> p k m", p=P))

        es_tiles = {}

        def qkv_mms(ps, lhsT_of_ko, rhs_of_ko, nn):
            """Contract over C with DoubleRow (fp8) or plain (bf16) matmuls."""
            if ATT_FP8:
                for kp in range(KC // 2):
                    nc.tensor.matmul(ps, lhsT_of_ko(2 * kp, 2),
                                     rhs_of_ko(2 * kp, 2),
                                     start=(kp == 0), stop=(kp == KC // 2 - 1),
                                     perf_mode=DR)
            else:
                for ko in range(KC):
                    nc.tensor.matmul(ps, lhsT_of_ko(ko, 1), rhs_of_ko(ko, 1),
                                     start=(ko == 0), stop=(ko == KC - 1))

        def emit_qkv(p, half):
            """q^T,k^T (bf16, pair-interleaved partitions) + V (token-major).

            half=0: first 512 tokens of q/k + V token tiles 0-3;
            half=1: the rest;  half=None: both.
            """
            halves = (0, 1) if half is None else (half,)
            for hf in halves:
                n0 = hf * 512
                for qk in range(2):
                    m = qk * KC + p  # wqkv column block (q: 0-5, k: 6-11)
                    ps = psum.tile([P, 512], F32, tag="mm", bufs=2, name="ps_qk")
                    qkv_mms(ps,
                            lambda ko, kn: wqkv_sb[:, ko:ko + kn,
                                                   m * P:(m + 1) * P],
                            lambda ko, kn: xlnT[:, ko:ko + kn, n0:n0 + 512],
                            512)
                    dst = qkT[:, qk * NPAIR + p, n0:n0 + 512]
                    s1 = (1.0 / WSCALE) if ATT_FP8 else 1.0
                    if bqkv_sb is not None:
                        nc.vector.tensor_scalar(out=dst, in0=ps, scalar1=s1,
                                                scalar2=bqkv_sb[:, m:m + 1],
                                                op0=ALU.mult, op1=ALU.add)
                    elif ATT_FP8:
                        nc.vector.tensor_scalar_mul(dst, ps, s1)
                    else:
                        nc.vector.tensor_copy(out=dst, in_=ps)
                # V token-major: V[tok, feat-pair] = x_ln @ wqkv[:, v-block]
                # (x8 scale retained in fp8 mode; denominator column folds it)
                c0 = 2 * C + 128 * p
                vps = psum.tile([P, 512], F32, tag="mm", bufs=2, name="ps_v")
                for ti in range(4):
                    t = hf * 4 + ti
                    qkv_mms(vps[:, ti * 128:(ti + 1) * 128],
                            lambda ko, kn: xlnT[:, ko:ko + kn,
                                                t * P:(t + 1) * P],
                            lambda ko, kn: wqkv_sb[:, ko:ko + kn, c0:c0 + 128],
                            128)
                vr = vps.rearrange("q (t h d) -> q t h d", t=4, d=HD)
                for hh in range(2):
                    dst = v_aug[:, 2 * p + hh, hf * 4:hf * 4 + 4, 0:HD]
                    src = vr[:, :, hh, :]
                    if bqkv_sb is not None:
                        bq = g_beta["bqkv_v"]
                        bsc = WSCALE if ATT_FP8 else 1.0
                        for ti in range(4):
                            bs = bq[:, 128 * p + hh * HD:128 * p + (hh + 1) * HD]
                            nc.vector.scalar_tensor_tensor(
                                out=dst[:, ti, :], in0=bs, scalar=bsc,
                                in1=src[:, ti, :], op0=ALU.mult, op1=ALU.add)
                    else:
                        nc.vector.tensor_copy(out=dst, in_=src)

        def emit_scores_kt(p, kt):
            """Row-packed score matmuls + exp for one key tile of pair p."""
            for hh in range(2):
                pb = hh * HD
                sc = psum.tile([P, 1024], F32, tag="sc", bufs=2,
                               name=f"sc{hh}")
                qT = qkT[pb:pb + HD, p, :]
                kT = qkT[pb:pb + HD, NPAIR + p, :]
                for j in range(2):
                    nc.tensor.matmul(sc[:, j * 512:(j + 1) * 512],
                                     kT[:, kt * P:(kt + 1) * P],
                                     qT[:, j * 512:(j + 1) * 512],
                                     start=True, stop=True)
                nc.scalar.activation(out=es_tiles[2 * p + hh][:, kt, :], in_=sc,
                                     func=AF.Exp, scale=0.125)

        def emit_av(p):
            """AV (+denominator row) for both heads of pair p, then normalize."""
            for hh in range(2):
                h = 2 * p + hh
                es = es_tiles[h]
                avs = {}
                for j in range(2):
                    av = psum.tile([HD + 1, 512], F32, tag="av", bufs=2,
                                   name=f"av{hh}{j}")
                    if ATT_FP8:
                        for ktp in range(NT // 2):
                            nc.tensor.matmul(
                                av, v_aug[:, h, 2 * ktp:2 * ktp + 2, 0:HD + 1],
                                es[:, 2 * ktp:2 * ktp + 2, j * 512:(j + 1) * 512],
                                start=(ktp == 0), stop=(ktp == NT // 2 - 1),
                                perf_mode=DR)
                    else:
                        for kt in range(NT):
                            nc.tensor.matmul(
                                av, v_aug[:, h, kt, 0:HD + 1],
                                es[:, kt, j * 512:(j + 1) * 512],
                                start=(kt == 0), stop=(kt == NT - 1))
                    avs[j] = av
                av_sb = att.tile([HD + 1, N], F32, tag="avsb", bufs=2,
                                 name=f"avsb{hh}")
                for j in range(2):
                    nc.vector.tensor_copy(out=av_sb[:, j * 512:(j + 1) * 512],
                                          in_=avs[j])
                # Softmax denominators: row HD holds 0.25*sum_k exp(S) (x8 V
                # scale / x32 attnT scale cancel). Broadcast via a DRAM
                # bounce, reciprocal, then normalize.
                rdram = dram.tile([1, N], F32, tag="rdram", bufs=2)
                nc.sync.dma_start(out=rdram, in_=av_sb[HD:HD + 1, :])
                rbc = att.tile([HD, N], F32, tag="rbc", bufs=2, name=f"rbc{hh}")
                rd = rdram[0, :]
                rbc_src = bass.AP(tensor=rd.tensor, offset=rd.offset,
                                  ap=[[0, HD]] + list(rd.ap))
                nc.sync.dma_start(out=rbc, in_=rbc_src)
                nc.vector.reciprocal_approx_fast(out=rbc, in_=rbc)
                if hh == 0:
                    # Even head lives on partitions 0-63 of attnT: write direct.
                    nc.vector.tensor_mul(out=attnT[0:HD, p, :],
                                         in0=av_sb[0:HD, :], in1=rbc)
                else:
                    bounce = att.tile([HD, N], a_dt, tag="bounce", bufs=2,
                                      name="bounce")
                    nc.vector.tensor_mul(out=bounce, in0=av_sb[0:HD, :], in1=rbc)
                    nc.sync.dma_start(out=attnT[HD:P, p, :], in_=bounce)

        # Phase 1 interleaved with QKV(0): LN1 tiles 0-3, first half of
        # QKV(0), LN1 tiles 4-7, second half.
        for t in range(NT):
            xln = layernorm_tile(ln1, x_sb[:, t, :], g_beta.get("g1"),
                                 g_beta.get("beta1"), "ln1")
            transpose_to(xln, xlnT, t)
            if t == 3 or t == 7:
                if t == 3:
                    for hh in range(2):
                        es_tiles[hh] = att.tile([P, NT, N], FP8 if ATT_FP8
                                                else BF16, tag="es", bufs=4,
                                                name=f"es{hh}")
                emit_qkv(0, half=(t - 3) // 4)

        for i in range(1, NPAIR + 2):
            # AV of pair i-2 first: its exp inputs are ready, so these
            # matmuls absorb PE time while ACT streams pair i-1's exps.
            if i >= 2:
                emit_av(i - 2)
            if i < NPAIR:
                for hh in range(2):
                    es_tiles[2 * i + hh] = att.tile([P, NT, N], FP8 if ATT_FP8
                                                    else BF16, tag="es", bufs=4,
                                                    name=f"es{2 * i + hh}")
                emit_qkv(i, half=None)
            if i <= NPAIR:
                for kt in range(NT):
                    emit_scores_kt(i - 1, kt)

        att.release()
        ln1.release()
        p1.release()

        # ---------------------------------------------------------------
        # Phase 3: proj + residual + LN2 -> x2_lnT (t 0-3), fc1 first half,
        # then t 4-7, fc1 second half, fc2, out.  wfc2/hT allocated on the
        # right above p2 (p2 stays allocated; SBUF fits both).
        # ---------------------------------------------------------------
        ln2 = tc.alloc_tile_pool(name="ln2", bufs=3, side="left")
        p5 = tc.alloc_tile_pool(name="p5", bufs=1, side="right")
        hT = p5.tile([P, KH, N], BF16)
        wfc2_sb = p5.tile([P, KH, C], BF16)
        nc.sync.dma_start(out=wfc2_sb,
                          in_=wfc2_d.ap().rearrange("(k p) m -> p k m", p=P))

        def emit_proj_ln2(t):
            for n0, nn in ((0, 512), (512, 256)):
                ps = psum.tile([P, 512], F32, tag="mm", bufs=2,
                               name="ps_pj")[:, :nn]
                qkv_mms(ps,
                        lambda ko, kn: attnT[:, ko:ko + kn, t * P:(t + 1) * P],
                        lambda ko, kn: wproj_sb[:, ko:ko + kn, n0:n0 + nn],
                        nn)
                xs = x_sb[:, t, n0:n0 + nn]
                if ATT_FP8:
                    nc.vector.scalar_tensor_tensor(
                        out=xs, in0=ps, scalar=1.0 / (32 * WSCALE), in1=xs,
                        op0=ALU.mult, op1=ALU.add)
                else:
                    nc.vector.tensor_add(out=xs, in0=xs, in1=ps)
                if "bproj" in g_beta:
                    nc.vector.tensor_add(out=xs, in0=xs,
                                         in1=g_beta["bproj"][:, n0:n0 + nn])
            xln = layernorm_tile(ln2, x_sb[:, t, :], g_beta.get("g2"),
                                 g_beta.get("beta2"), "ln2")
            transpose_to(xln, x2lnT, t)

        gelu_scale = (1.0 / WSCALE) if FC1_FP8 else 1.0

        def emit_fc1(n0):
            for m in range(KH):
                ps = psum.tile([P, 512], F32, tag="mm", bufs=2, name="ps_f1")
                if FC1_FP8:
                    for kp in range(KC // 2):
                        nc.tensor.matmul(
                            ps, wfc1_sb[:, 2 * kp:2 * kp + 2, m * P:(m + 1) * P],
                            x2lnT[:, 2 * kp:2 * kp + 2, n0:n0 + 512],
                            start=(kp == 0), stop=(kp == KC // 2 - 1),
                            perf_mode=DR)
                else:
                    for ko in range(KC):
                        nc.tensor.matmul(ps, wfc1_sb[:, ko, m * P:(m + 1) * P],
                                         x2lnT[:, ko, n0:n0 + 512],
                                         start=(ko == 0), stop=(ko == KC - 1))
                bias = bfc1_sb[:, m:m + 1] if bfc1_sb is not None else 0.0
                nc.scalar.activation(out=hT[:, m, n0:n0 + 512], in_=ps,
                                     func=AF.Gelu, bias=bias, scale=gelu_scale)

        def emit_fc2(t):
            for n0, nn in ((0, 512), (512, 256)):
                ps = psum.tile([P, 512], F32, tag="mm", bufs=2,
                               name="ps_f2")[:, :nn]
                for ko in range(KH):
                    nc.tensor.matmul(ps, hT[:, ko, t * P:(t + 1) * P],
                                     wfc2_sb[:, ko, n0:n0 + nn],
                                     start=(ko == 0), stop=(ko == KH - 1))
                xs = x_sb[:, t, n0:n0 + nn]
                nc.vector.tensor_add(out=xs, in0=xs, in1=ps)
                if "bfc2" in g_beta:
                    nc.vector.tensor_add(out=xs, in0=xs,
                                         in1=g_beta["bfc2"][:, n0:n0 + nn])
            nc.sync.dma_start(out=out_d.ap()[t * P:(t + 1) * P, :],
                              in_=x_sb[:, t, :])

        for t in range(4):
            emit_proj_ln2(t)
        emit_fc1(0)
        for t in range(4, NT):
            emit_proj_ln2(t)
        emit_fc1(512)
        for t in range(NT):
            emit_fc2(t)

        ln2.release()
        p5.release()
        p2.release()
        persist.release()
        dram.release()
        psum.release()

    nc.compile()
    return nc


def _prep(inputs):
    """Host-side prep: shard x over B, cast weights, compute flag gates."""
    f = {k: np.asarray(v) for k, v in inputs.items()}
    bf = ml_dtypes.bfloat16
    f8 = getattr(ml_dtypes, "float8_e4m3fn", None) or ml_dtypes.float8_e4m3

    flags = (
        bool(np.any(f["b_qkv"])),
        not np.all(f["g1"] == 1.0),
        bool(np.any(f["beta1"])),
        not np.all(f["g2"] == 1.0),
        bool(np.any(f["beta2"])),
        bool(np.any(f["b_fc1"])),
        bool(np.any(f["b_proj"])),
        bool(np.any(f["b_fc2"])),
    )
    (use_bqkv, use_g1, use_beta1, use_g2, use_beta2, use_bfc1, use_bproj,
     use_bfc2) = flags

    def wcast(w, fp8_on):
        if fp8_on:
            return np.ascontiguousarray((w * WSCALE).astype(f8))
        return np.ascontiguousarray(w.astype(bf))

    common = {
        "wqkv": wcast(f["w_qkv"], ATT_FP8),
        "wproj": wcast(f["w_proj"], ATT_FP8),
        "wfc1": wcast(f["w_fc1"], FC1_FP8),
        "wfc2": np.ascontiguousarray(f["w_fc2"].astype(bf)),
    }
    for name, key, use in (
        ("bqkv", "b_qkv", use_bqkv), ("g1", "g1", use_g1),
        ("beta1", "beta1", use_beta1), ("g2", "g2", use_g2),
        ("beta2", "beta2", use_beta2), ("bfc1", "b_fc1", use_bfc1),
        ("bproj", "b_proj", use_bproj), ("bfc2", "b_fc2", use_bfc2),
    ):
        if use:
            common[name] = np.ascontiguousarray(f[key].astype(np.float32))

    x = f["x"].astype(np.float32)
    in_maps = [dict(common, x=np.ascontiguousarray(x[i])) for i in range(B)]
    return flags, in_maps


LAST_RESULT = None


def kernel(**inputs):
    global LAST_RESULT
    flags, in_maps = _prep(inputs)
    if flags not in _cache:
        _cache[flags] = _build(flags)
    nc = _cache[flags]
    res = bass_utils.run_bass_kernel_spmd(nc, in_maps, core_ids=list(range(B)))
    LAST_RESULT = res
    out = np.stack([r["out"] for r in res.results], axis=0)
    return out.astype(np.float32)
